# revision 13
# baseline (speedup 1.0000x reference)
"""Trainium2 Bass kernel for nn_COSLoss_52905407152941.

Loss = cos_texture + mse_silhoue factors into per-pixel n-reductions:
    S_pm[p]  = sum_n pd_mask[n,p]
    S_gm[p]  = sum_n gt_mask[n,p]
    S_jt[p]  = sum_n pd_mask[n,p]*gt_mask[n,p]
    loss = 1 + sum_p[ pd4^2*S_pm + gt4^2*S_gm - (dot3 + 2*pd4*gt4)*S_jt ] / (b*n*h*w)

Sharding: pure data parallel, batch i -> core i (8 batches, 8 cores).

Per core the dominant cost is streaming the two int32 {0,1} masks
(8 MB each) from HBM.  The masks are fed to the device as raw bytes
reinterpreted as fp8e4 (int32 1 == bytes 01 00 00 00 == fp8 2^-9 at
byte 0, zeros elsewhere), so the n-reduction runs on the otherwise-idle
TensorEngine as identity-weight matmuls accumulating in PSUM with NO
cast pass: S_psum = 2^-9 * S exactly (powers of two, fp32 PSUM).  The
joint mask product is one strided DVE multiply per 4-plane group
(fp8*fp8 -> bf16 2^-18, exact), reduced the same way.  The final
combine (3 multiplies + 3 free-dim reduces) runs on DVE; each core
emits a [128,3] fp32 partial and the host applies the exact 2^9/2^18
rescales and finishes in float64.  The last mask group streams
per-plane so only one 512-wide product sits on the critical tail.
"""

import sys

if "/opt/trn_rl_repo" not in sys.path:
    sys.path.insert(0, "/opt/trn_rl_repo")

import numpy as np
import ml_dtypes

B, C, N, H, W = 8, 4, 32, 256, 256
PIX = H * W          # 65536 pixels per plane
K = 128              # SBUF partitions
F = PIX // K         # 512 free elems per partition per plane
NCORES = 8
PLANES_PER_GROUP = 4
GROUPS = N // PLANES_PER_GROUP  # 8

_CACHE = {}


def build_nc(reps=1):
    """Build + compile the per-core program. reps>1 wraps the body in a
    For_i loop (used only by test.py for slope timing)."""
    import concourse.bacc as bacc
    import concourse.tile as tile
    import concourse.mybir as mybir

    fp8 = mybir.dt.float8e4
    bf16 = mybir.dt.bfloat16
    f32 = mybir.dt.float32
    mult = mybir.AluOpType.mult
    add = mybir.AluOpType.add

    nc = bacc.Bacc("TRN2", target_bir_lowering=False, debug=False)
    pd = nc.dram_tensor("pd", (C, PIX), f32, kind="ExternalInput").ap()
    gt = nc.dram_tensor("gt", (C, PIX), f32, kind="ExternalInput").ap()
    pm = nc.dram_tensor("pm8", (N, 4 * PIX), fp8, kind="ExternalInput").ap()
    gm = nc.dram_tensor("gm8", (N, 4 * PIX), fp8, kind="ExternalInput").ap()
    eye8 = nc.dram_tensor("eye8", (K, K), fp8, kind="ExternalInput").ap()
    eyeb = nc.dram_tensor("eyeb", (K, K), bf16, kind="ExternalInput").ap()
    # columns: [sum (dot3+2 pd4 gt4) S_jt*2^-18, sum pd4^2 S_pm*2^-9, sum gt4^2 S_gm*2^-9]
    out = nc.dram_tensor("partial", (K, 3), f32, kind="ExternalOutput").ap()

    with tile.TileContext(nc) as tc:

        def body(consts, masks, prods, pgp, tmps, psum):
            X = mybir.AxisListType.X
            G = PLANES_PER_GROUP
            LG = GROUPS - 1  # last group, streamed per-plane to shrink the tail

            eye8_t = consts.tile([K, K], fp8, tag="eye8")
            eyeb_t = consts.tile([K, K], bf16, tag="eyeb")
            pd_t = pgp.tile([K, C, F], f32, tag="pd")
            gt_t = pgp.tile([K, C, F], f32, tag="gt")

            s_pm = psum.tile([K, F], f32, tag="s_pm")
            s_gm = psum.tile([K, F], f32, tag="s_gm")
            s_jt = psum.tile([K, F], f32, tag="s_jt")

            # group 0 masks first so HBM streaming starts immediately
            group_tiles = {}
            def load_group(g):
                pm_t = masks.tile([K, G, 4 * F], fp8, tag="pm")
                gm_t = masks.tile([K, G, 4 * F], fp8, tag="gm")
                nc.sync.dma_start(
                    pm_t[:], pm[g * G:(g + 1) * G].rearrange("n (k f) -> k n f", k=K))
                nc.sync.dma_start(
                    gm_t[:], gm[g * G:(g + 1) * G].rearrange("n (k f) -> k n f", k=K))
                group_tiles[g] = (pm_t, gm_t)

            load_group(0)
            nc.sync.dma_start(eye8_t[:], eye8[:])
            nc.sync.dma_start(eyeb_t[:], eyeb[:])
            nc.sync.dma_start(pd_t[:], pd.rearrange("c (k f) -> k c f", k=K))
            nc.sync.dma_start(gt_t[:], gt.rearrange("c (k f) -> k c f", k=K))

            # prep everything that only needs pd/gt, in dedicated tiles, so it
            # all runs while the masks stream
            w_t = tmps.tile([K, F], f32, tag="w_t")        # dot3 + 2*pd4*gt4
            t_b = tmps.tile([K, F], f32, tag="t_b")
            sq_pd = tmps.tile([K, F], f32, tag="sq_pd")    # pd4^2
            sq_gt = tmps.tile([K, F], f32, tag="sq_gt")    # gt4^2
            tr_pm = tmps.tile([K, F], f32, tag="tr_pm")
            tr_gm = tmps.tile([K, F], f32, tag="tr_gm")
            tr_jt = tmps.tile([K, F], f32, tag="tr_jt")
            out_t = tmps.tile([K, 3], f32, tag="out_t")

            nc.vector.tensor_tensor(w_t[:], pd_t[:, 0, :], gt_t[:, 0, :], op=mult)
            nc.vector.tensor_tensor(t_b[:], pd_t[:, 1, :], gt_t[:, 1, :], op=mult)
            nc.vector.tensor_tensor(w_t[:], w_t[:], t_b[:], op=add)
            nc.vector.tensor_tensor(t_b[:], pd_t[:, 2, :], gt_t[:, 2, :], op=mult)
            nc.vector.tensor_tensor(w_t[:], w_t[:], t_b[:], op=add)   # dot3
            nc.vector.tensor_tensor(t_b[:], pd_t[:, 3, :], gt_t[:, 3, :], op=mult)
            nc.vector.tensor_scalar_mul(t_b[:], t_b[:], 2.0)
            nc.vector.tensor_tensor(w_t[:], w_t[:], t_b[:], op=add)   # +2*pd4*gt4
            nc.vector.tensor_tensor(sq_pd[:], pd_t[:, 3, :], pd_t[:, 3, :], op=mult)
            nc.vector.tensor_tensor(sq_gt[:], gt_t[:, 3, :], gt_t[:, 3, :], op=mult)

            def process_group(g):
                pm_t, gm_t = group_tiles.pop(g)
                pm_v = pm_t[:].rearrange("k n (f four) -> k n f four", four=4)
                gm_v = gm_t[:].rearrange("k n (f four) -> k n f four", four=4)
                prod = prods.tile([K, G, F], bf16, tag="prod")
                nc.vector.tensor_tensor(
                    prod[:], pm_v[:, :, :, 0], gm_v[:, :, :, 0], op=mult)
                for j in range(G):
                    n = g * G + j
                    first = n == 0
                    nc.tensor.matmul(s_pm[:], eye8_t[:], pm_v[:, j, :, 0],
                                     start=first, stop=False)
                    nc.tensor.matmul(s_gm[:], eye8_t[:], gm_v[:, j, :, 0],
                                     start=first, stop=False)
                    nc.tensor.matmul(s_jt[:], eyeb_t[:], prod[:, j, :],
                                     start=first, stop=False)

            for g in range(GROUPS):
                if g == LG:
                    break
                if g + 1 < LG:
                    load_group(g + 1)
                process_group(g)

            # last group: per-plane DMAs/products; close s_pm/s_gm as early as
            # possible so their combines overlap the remaining jt work
            lg_pm = masks.tile([K, G, 4 * F], fp8, tag="pm")
            lg_gm = masks.tile([K, G, 4 * F], fp8, tag="gm")
            lg_prod = prods.tile([K, G, F], bf16, tag="prod")
            pm_v = lg_pm[:].rearrange("k n (f four) -> k n f four", four=4)
            gm_v = lg_gm[:].rearrange("k n (f four) -> k n f four", four=4)
            # per-plane interleaved pm/gm: products pipeline with DMA arrivals
            for j in range(G):
                n = LG * G + j
                nc.sync.dma_start(
                    lg_pm[:, j, :], pm[n].rearrange("(k f) -> k f", k=K))
                nc.sync.dma_start(
                    lg_gm[:, j, :], gm[n].rearrange("(k f) -> k f", k=K))
            for j in range(G):
                last = j == G - 1
                nc.tensor.matmul(s_pm[:], eye8_t[:], pm_v[:, j, :, 0],
                                 start=False, stop=last)
                nc.tensor.matmul(s_gm[:], eye8_t[:], gm_v[:, j, :, 0],
                                 start=False, stop=last)
                nc.vector.tensor_tensor(
                    lg_prod[:, j, :], pm_v[:, j, :, 0], gm_v[:, j, :, 0], op=mult)
                nc.tensor.matmul(s_jt[:], eyeb_t[:], lg_prod[:, j, :],
                                 start=False, stop=last)

            nc.vector.tensor_tensor(tr_pm[:], sq_pd[:], s_pm[:], op=mult)
            nc.vector.tensor_reduce(out_t[:, 1:2], tr_pm[:], axis=X, op=add)
            nc.vector.tensor_tensor(tr_gm[:], sq_gt[:], s_gm[:], op=mult)
            nc.vector.tensor_reduce(out_t[:, 2:3], tr_gm[:], axis=X, op=add)
            nc.vector.tensor_tensor(tr_jt[:], w_t[:], s_jt[:], op=mult)
            nc.vector.tensor_reduce(out_t[:, 0:1], tr_jt[:], axis=X, op=add)
            nc.sync.dma_start(out[:], out_t[:])

        import contextlib

        with contextlib.ExitStack() as ctx:
            pools = (
                ctx.enter_context(tc.tile_pool(name="consts", bufs=1)),
                ctx.enter_context(tc.tile_pool(name="masks", bufs=4)),
                ctx.enter_context(tc.tile_pool(name="prods", bufs=3)),
                ctx.enter_context(tc.tile_pool(name="pg", bufs=1)),
                ctx.enter_context(tc.tile_pool(name="tmps", bufs=1)),
                ctx.enter_context(
                    tc.tile_pool(name="psum", bufs=1, space="PSUM")),
            )
            if reps == 1:
                body(*pools)
            else:
                with tc.For_i(0, reps):
                    body(*pools)

    nc.compile()
    return nc


def _shard_inputs(pd, gt, pd_mask, gt_mask):
    pd = np.ascontiguousarray(np.asarray(pd, dtype=np.float32))
    gt = np.ascontiguousarray(np.asarray(gt, dtype=np.float32))
    pd_mask = np.ascontiguousarray(np.asarray(pd_mask, dtype=np.int32))
    gt_mask = np.ascontiguousarray(np.asarray(gt_mask, dtype=np.int32))
    eye8 = np.eye(K, dtype=ml_dtypes.float8_e4m3)
    eyeb = np.eye(K, dtype=ml_dtypes.bfloat16)
    in_maps = []
    for i in range(NCORES):
        in_maps.append({
            "pd": pd[i].reshape(C, PIX),
            "gt": gt[i].reshape(C, PIX),
            "pm8": pd_mask[i].reshape(N, PIX).view(ml_dtypes.float8_e4m3),
            "gm8": gt_mask[i].reshape(N, PIX).view(ml_dtypes.float8_e4m3),
            "eye8": eye8,
            "eyeb": eyeb,
        })
    return in_maps


def _unshard(results):
    total = 0.0
    for r in results:
        cols = r["partial"].astype(np.float64).sum(axis=0)  # (3,)
        total += (2.0 ** 9) * (cols[1] + cols[2]) - (2.0 ** 18) * cols[0]
    return np.float32(1.0 + total / (B * N * PIX))


def kernel(pd, gt, pd_mask, gt_mask):
    if "nc" not in _CACHE:
        _CACHE["nc"] = build_nc()
    from concourse import bass2jax
    in_maps = _shard_inputs(pd, gt, pd_mask, gt_mask)
    try:
        results = bass2jax.run_bass_via_pjrt(_CACHE["nc"], in_maps, n_cores=NCORES)
    except Exception:
        # one retry for transient axon-link hiccups
        results = bass2jax.run_bass_via_pjrt(_CACHE["nc"], in_maps, n_cores=NCORES)
    return _unshard(results)
